# revision 12
# baseline (speedup 1.0000x reference)
"""LBQuantization Trainium2 Bass kernel (nn_LBQuantization_35021163331684).

Math per channel (C = B*c = 96, HW = 512*512 px):
    mn, mx = min(x_ch), max(x_ch)
    t_j = rp_j * (mx - mn) + mn   (rp pre-sorted on host, j = 1..7)
    out = largest v in {mn, t_1..t_7} with v <= pixel

v2 over the tuned baseline: the per-channel min/max no longer costs a
full DVE pass. TRN2's Pool engine cannot reduce along the free dim
(TensorReduce/Pool/TensorTensor/scan are all ISA-illegal on Pool), but
partition_all_reduce (GPSIMD ucode) CAN reduce across partitions at
full width. So for channels >= 2:
    ACT:  xn = -x (exact sign flip) into the other half of a concat tile
    Pool: partition_all_reduce(max) over each half -> per-COLUMN max /
          -min, identical on all 128 partitions
    DMA:  reshape one partition's 2048 column-reduces to [128, 16]
          (a 2 us-latency, ~56 ns-occupancy SBUF->SBUF copy)
    DVE:  tensor_reduce over 16 cols (~0.16 us, replacing the 2.2 us
          fused min/max scan) + the baseline [P,2] all-reduce
Channels 0-1 keep the DVE minmax scan: the Pool route has ~10 us of
pipeline latency which the ramp cannot hide, and the DVE is idle there
anyway.

DVE per channel: 3 exact select passes (SEL3N/SEL2C/SEL2C: 2-3
compare-thresholds per op is the v3 custom-DVE slot limit) + the tiny
column reduce + a share of the mn-patch at the 4x bf16 tensor_scalar
rate. Pool: the two 2048-wide all-reduces + the rest of the patch.
Engines balance at ~7.3 us/channel vs the baseline's 8.9 (DVE-bound).

All compares are exact fp32 against exact thresholds, so bucketing
matches the reference exactly; only output VALUES carry bf16 rounding
(rel err ~3e-3, gate 2e-2). bf16 also halves write DMA.

Schedule: software-pipelined two deep -- channel c's selects overlap
channel c+1's diag/reduce/threshold prep and channel c+2's load/negate/
all-reduces. Channel 0 ramps in column chunks; the last channel drains
in shrinking slices patched on the DVE.

Sharding: fully data-parallel, 12 channels/core x 8 cores, no
collectives.
"""

import sys

if "/opt/trn_rl_repo" not in sys.path:
    sys.path.insert(0, "/opt/trn_rl_repo")

import numpy as np

N_CORES = 8
B, CC, H, W = 32, 3, 512, 512
C_TOTAL = B * CC          # 96
C_PER = C_TOTAL // N_CORES  # 12
P = 128
FD = (H * W) // P         # 2048
CH = FD // P              # 16 cols per partition after the diag reshape
R = 8                     # region_num
N_OLD = 3                 # channels using the DVE minmax scan (ramp)
Q_POOL = 936              # mn-patch columns handled by Pool (rest DVE 4x)

_CACHE: dict = {}


# --------------------------------------------------------------------------- #
# Custom DVE ops (SEL3N / SEL2C / AFFINE / MINMAX, from the tuned baseline)
# --------------------------------------------------------------------------- #
def _register_ops():
    if "ops" in _CACHE:
        return _CACHE["ops"]
    from concourse import dve_ops
    from concourse.dve_spec import (
        C0,
        C1,
        C3,
        AluOp,
        MaxNeg,
        Spec,
        Src0,
        Src1,
        Zero,
        _spill_c3_to_src1,
        lower,
        scan,
        select,
    )
    from concourse.dve_uop import AluInp, DveOpSpec

    def mk(name, spec):
        if name in dve_ops._SUB_OPCODE_FOR_NAME:
            return next(op for op in dve_ops.OPS if op.name == name)
        row = dve_ops._CUSTOM_DVE_ROW_BASE + len(dve_ops.OPS)
        assert row < 0x20, "custom DVE opcode rows exhausted"
        dve_ops._SUB_OPCODE_FOR_NAME[name] = row
        shas = {}
        for ver in ("v3", "v4"):
            try:
                shas[ver] = DveOpSpec(
                    name=name,
                    opcode=row,
                    uops=lower(spec, ver=ver),
                    rd1_en=dve_ops.has_src1(spec),
                ).sha(ver)
            except ValueError:
                pass
        assert "v3" in shas, f"{name}: v3 lowering failed"
        op = dve_ops.DveOp(name, spec, subdim=False, uops_sha=shas)
        dve_ops.OPS.append(op)
        dve_ops.CUSTOM_DVE_SPECS[name] = spec
        return op

    # r = x>=t_c ? t_c : (x>=t_b ? t_b : (x>=t_a ? t_a : -FLT_MAX))
    # [t_c via C3 -> in1 [P,1]]
    sel3 = mk(
        "LBQ_SEL3N",
        Spec(
            body=_spill_c3_to_src1(
                select(
                    Src0 >= C3,
                    C3,
                    select(Src0 >= C1, C1, select(Src0 >= C0, C0, MaxNeg)),
                )
            ),
            reference=lambda in0, in1, c0, c1, c2: np.where(
                in0 >= in1, in1,
                np.where(
                    in0 >= c1, c1,
                    np.where(in0 >= c0, c0, np.float32(-3.4028235e38)),
                ),
            ).astype(np.float32),
        ),
    )
    # r = x>=t_b ? t_b : (x>=t_a ? t_a : carry)   [carry via Src1 [P,N]]
    sel2c = mk(
        "LBQ_SEL2C",
        Spec(
            body=select(Src0 >= C1, C1, select(Src0 >= C0, C0, Src1)),
            reference=lambda in0, in1, c0, c1, c2: np.where(
                in0 >= c1, c1, np.where(in0 >= c0, c0, in1)
            ).astype(np.float32),
        ),
    )
    # pos = rp*rng + mn (exact mul-then-add on the DVE datapath)
    affine = mk(
        "LBQ_AFFINE",
        Spec(
            body=Src0 * C0 + C1,
            reference=lambda in0, in1, c0, c1, c2: (
                in0.astype(np.float32) * c0 + c1
            ).astype(np.float32),
        ),
    )

    # Single-pass dual min/max (ramp channels): out stream = running max of
    # -x drained through a stride-0 AP (only -min lands); the accum stage is
    # rewired post-lowering to fold raw Src0 -> per-lane max.
    def _minmax_ref(in0, in1, c0, c1, c2):
        x = in0.astype(np.float32)
        negmins = np.maximum.accumulate(np.maximum(-x, np.float32(c0)), axis=-1)
        mx = x.reshape(x.shape[0], -1).max(axis=-1, keepdims=True)
        return negmins, np.maximum(mx, np.float32(-3.4028235e38))

    mm_name = "LBQ_MINMAX"
    if mm_name not in dve_ops._SUB_OPCODE_FOR_NAME:
        mm_spec = Spec(
            body=scan(AluOp.MAX, Zero - Src0, init=C0),
            accum=AluOp.MAX,
            reference=_minmax_ref,
        )
        row = dve_ops._CUSTOM_DVE_ROW_BASE + len(dve_ops.OPS)
        assert row < 0x20
        dve_ops._SUB_OPCODE_FOR_NAME[mm_name] = row
        uops = lower(mm_spec, ver="v3")
        steady = uops[-1]
        acc_st = None
        src0_lane = None
        for st, dp in enumerate(steady.datapath_config):
            if int(dp.alu_out_a_enable):
                assert dp.op == AluOp.MAX and dp.alu_src1 == AluInp.PREV_ALU_OUT
                acc_st = st
                break
        for lane_idx in range(1, 7):
            if int(steady.inp_enable[lane_idx]) and steady.inp[lane_idx].name == "SRC_0":
                src0_lane = lane_idx - 1
                break
        assert acc_st is not None and src0_lane is not None, (acc_st, src0_lane)
        steady.datapath_config[acc_st].alu_src1 = AluInp(
            int(AluInp.PREV_DELAY_0) + src0_lane
        )
        compiled = DveOpSpec(name=mm_name, opcode=row, uops=uops, rd1_en=False)
        minmax = dve_ops.DveOp(
            mm_name,
            mm_spec,
            subdim=False,
            uops_sha={"v3": compiled.sha("v3")},
        )
        dve_ops._COMPILE_CACHE[(mm_name, "v3")] = compiled
        dve_ops.OPS.append(minmax)
        dve_ops.CUSTOM_DVE_SPECS[mm_name] = mm_spec
    else:
        minmax = next(op for op in dve_ops.OPS if op.name == mm_name)

    _CACHE["ops"] = (sel3, sel2c, affine, minmax)
    return _CACHE["ops"]


# --------------------------------------------------------------------------- #
# Bass module (SPMD: same program on all 8 cores, different data)
# --------------------------------------------------------------------------- #
def _build_module():
    if "nc" in _CACHE:
        return _CACHE["nc"]
    import concourse.bacc as bacc
    import concourse.bass as bass
    import concourse.bass_isa as bass_isa
    import concourse.tile as tile
    from concourse import mybir

    SEL3, SEL2C, AFFINE, MINMAX = _register_ops()
    f32 = mybir.dt.float32
    bf16 = mybir.dt.bfloat16
    FLT_MAX = 3.4028234663852886e38

    nc = bacc.Bacc("TRN2", target_bir_lowering=False, name="lbq5")
    x_d = nc.dram_tensor("x", [C_PER, P, FD], f32, kind="ExternalInput")
    rp_d = nc.dram_tensor("rp", [C_PER, R - 1], f32, kind="ExternalInput")
    y_d = nc.dram_tensor("y", [C_PER, P, FD], bf16, kind="ExternalOutput")

    with tile.TileContext(nc) as tc:
        with (
            tc.tile_pool(name="xp", bufs=5) as xp,
            tc.tile_pool(name="cp", bufs=3) as cp,
            tc.tile_pool(name="wp", bufs=2) as wp,
            tc.tile_pool(name="sp", bufs=1) as sp,
            tc.tile_pool(name="op", bufs=3) as op_,
        ):
            # rp [12,7] DRAM -> one SBUF row -> gpsimd broadcast to [128, 84]
            rp_b = sp.tile([P, C_PER, R - 1], f32, tag="rp_b")

            def emit_rp():
                rp_row = sp.tile([1, C_PER * (R - 1)], f32, tag="rp_row")
                rp_ap = rp_d[:, :]
                nc.sync.dma_start(
                    out=rp_row,
                    in_=bass.AP(
                        tensor=rp_ap.tensor,
                        offset=rp_ap.offset,
                        ap=[[0, 1], [1, C_PER * (R - 1)]],
                    ),
                )
                nc.gpsimd.partition_broadcast(
                    rp_b.rearrange("p c r -> p (c r)"), rp_row, channels=P
                )

            def minmax_sink(dst_negmin, dst_max, src, fd):
                sink = bass.AP(
                    tensor=dst_negmin.tensor,
                    offset=dst_negmin.offset,
                    ap=[list(dst_negmin.ap[0]), [0, fd]],
                )
                nc.vector._custom_dve(
                    MINMAX, out=sink, in0=src,
                    s0=-FLT_MAX, accum_out=dst_max,
                )

            # ------------------------------------------------------------- #
            # per-channel state
            # xc[c]: [P, 2, FD] concat tile; half 1 = x (DMA'd), half 0 = -x
            # ------------------------------------------------------------- #
            state: dict = {}

            def ld(c):
                """DMA channel c into the x half of a fresh concat tile."""
                xc = xp.tile([P, 2, FD], f32, tag="xc")
                st = state.setdefault(c, {})
                st["xc"] = xc
                nc.sync.dma_start(out=xc[:, 1, :], in_=x_d[c])
                return st

            def ld_scan0():
                """Channel 0 gates the ramp: interleave load chunks with DVE
                min/max scan chunks so the first compute starts early."""
                xc = xp.tile([P, 2, FD], f32, tag="xc")
                st = state.setdefault(0, {})
                st["xc"] = xc
                xt = xc[:, 1, :]
                pm = sp.tile([P, 2], f32, tag="pm0")
                bounds = [0, 448, 1024, 1600, FD]
                n_ck = len(bounds) - 1
                pm8 = sp.tile([P, 2, n_ck], f32, tag="pm_ck")
                for i in range(n_ck):
                    sl = slice(bounds[i], bounds[i + 1])
                    nc.sync.dma_start(out=xt[:, sl], in_=x_d[0][:, sl])
                    minmax_sink(
                        pm8[:, 0, i : i + 1], pm8[:, 1, i : i + 1],
                        xt[:, sl], bounds[i + 1] - bounds[i],
                    )
                nc.vector.tensor_reduce(
                    out=pm[:, 0:1], in_=pm8[:, 0, :],
                    axis=mybir.AxisListType.X, op=mybir.AluOpType.max,
                )
                nc.vector.tensor_reduce(
                    out=pm[:, 1:2], in_=pm8[:, 1, :],
                    axis=mybir.AxisListType.X, op=mybir.AluOpType.max,
                )
                ar = sp.tile([P, 2], f32, tag="ar0")
                nc.gpsimd.partition_all_reduce(
                    ar, pm, P, bass_isa.ReduceOp.max
                )
                st["ar"] = ar

            def old_scan(c):
                """Ramp path: DVE fused min/max scan + [P,2] all-reduce."""
                st = state[c]
                xt = st["xc"][:, 1, :]
                pm = sp.tile([P, 2], f32, tag=f"pm{c}")
                minmax_sink(pm[:, 0:1], pm[:, 1:2], xt, FD)
                ar = sp.tile([P, 2], f32, tag=f"ar{c}")
                nc.gpsimd.partition_all_reduce(
                    ar, pm, P, bass_isa.ReduceOp.max
                )
                st["ar"] = ar

            def neg(c):
                """ACT: exact negate of x into the other concat half."""
                xc = state[c]["xc"]
                nc.scalar.activation(
                    out=xc[:, 0, :], in_=xc[:, 1, :],
                    func=mybir.ActivationFunctionType.Copy, scale=-1.0,
                )

            def allred(c):
                """Pool: per-column max of x and of -x across partitions."""
                st = state[c]
                xc = st["xc"]
                cm = cp.tile([P, 2, FD], f32, tag="cm")
                st["cm"] = cm
                nc.gpsimd.partition_all_reduce(
                    cm[:, 1, :], xc[:, 1, :], P, bass_isa.ReduceOp.max
                )
                nc.gpsimd.partition_all_reduce(
                    cm[:, 0, :], xc[:, 0, :], P, bass_isa.ReduceOp.max
                )

            def diag(c):
                """Reshape-DMA: spread one partition's column reduces across
                the 128 partitions (16 cols each)."""
                cm = state[c]["cm"]
                dg = sp.tile([P, 2, CH], f32, tag=f"dg{c % 3}")
                state[c]["dg"] = dg
                nc.sync.dma_start(out=dg[:, 0, :], in_=cm[0:1, 0, :])
                nc.sync.dma_start(out=dg[:, 1, :], in_=cm[0:1, 1, :])

            def red(c):
                """DVE: fold each partition's 2x16 chunk; Pool: combine."""
                st = state[c]
                pm = sp.tile([P, 2], f32, tag=f"pm{c}")
                nc.vector.tensor_reduce(
                    out=pm, in_=st["dg"], axis=mybir.AxisListType.X,
                    op=mybir.AluOpType.max,
                )
                ar = sp.tile([P, 2], f32, tag=f"ar{c}")
                nc.gpsimd.partition_all_reduce(
                    ar, pm, P, bass_isa.ReduceOp.max
                )
                st["ar"] = ar

            def thr_act(c):
                """ACT half of threshold prep: mn = -(-mn), rng = mx + (-mn)."""
                st = state[c]
                ar = st["ar"]
                thr = sp.tile([P, R], f32, tag=f"thr{c}")
                rng = sp.tile([P, 1], f32, tag=f"rng{c}")
                nc.scalar.activation(
                    out=thr[:, 0:1], in_=ar[:, 0:1],
                    func=mybir.ActivationFunctionType.Copy, scale=-1.0,
                )
                nc.scalar.activation(
                    out=rng, in_=ar[:, 1:2],
                    func=mybir.ActivationFunctionType.Identity,
                    bias=ar[:, 0:1], scale=1.0,
                )
                st["thr"] = thr
                st["rng"] = rng

            def affine(c):
                """DVE: pos = rp*rng + mn (exact mul-then-add)."""
                st = state[c]
                thr = st["thr"]
                nc.vector._custom_dve(
                    AFFINE, out=thr[:, 1:R], in0=rp_b[:, c, :],
                    s0=st["rng"][:, 0:1], s1=thr[:, 0:1],
                )

            # ------------------------------------------------------------- #
            # select chain + patch + out-DMA
            # ------------------------------------------------------------- #
            def process(c, hook1=None, hook2=None, hook3=None):
                st = state[c]
                xt = st["xc"][:, 1, :]
                thr = st["thr"]
                c1t = wp.tile([P, FD], f32, tag="c1")
                c2t = wp.tile([P, FD], f32, tag="c2")
                rbt = wp.tile([P, FD], bf16, tag="rb")
                ot = op_.tile([P, FD], bf16, tag="out")

                first = [True]

                def body(lo, hi, q_pool=0):
                    sl = slice(lo, hi)
                    nc.vector._custom_dve(
                        SEL3, out=c1t[:, sl], in0=xt[:, sl], in1=thr[:, 3:4],
                        s0=thr[:, 1:2], s1=thr[:, 2:3],
                    )
                    if first[0] and hook1 is not None:
                        hook1()
                    nc.vector._custom_dve(
                        SEL2C, out=c2t[:, sl], in0=xt[:, sl], in1=c1t[:, sl],
                        s0=thr[:, 4:5], s1=thr[:, 5:6],
                    )
                    if first[0] and hook2 is not None:
                        hook2()
                    nc.vector._custom_dve(
                        SEL2C, out=rbt[:, sl], in0=xt[:, sl], in1=c2t[:, sl],
                        s0=thr[:, 6:7], s1=thr[:, 7:8],
                    )
                    if first[0] and hook3 is not None:
                        hook3()
                    first[0] = False
                    # patch: out = max(r, mn); bf16 in/out. Split between the
                    # Pool engine and the DVE's 4x bf16 tensor_scalar rate.
                    mid = min(lo + q_pool, hi)
                    if mid > lo:
                        nc.gpsimd.tensor_scalar_max(
                            ot[:, lo:mid], rbt[:, lo:mid], thr[:, 0:1]
                        )
                        nc.sync.dma_start(
                            out=y_d[c][:, lo:mid], in_=ot[:, lo:mid]
                        )
                    if hi > mid:
                        nc.vector.tensor_scalar_max(
                            ot[:, mid:hi], rbt[:, mid:hi], thr[:, 0:1]
                        )
                        nc.sync.dma_start(
                            out=y_d[c][:, mid:hi], in_=ot[:, mid:hi]
                        )

                if c == C_PER - 1:
                    # the last channel gates the kernel tail: drain in
                    # shrinking slices, all patched on the DVE at 4x
                    t11 = [0, 1024, 1664, FD]
                    for i in range(len(t11) - 1):
                        body(t11[i], t11[i + 1])
                elif c < N_OLD:
                    # ramp channels: Pool is idle, patch fully there
                    body(0, FD, q_pool=FD)
                else:
                    body(0, FD, q_pool=Q_POOL)

            # ------------------------------------------------------------- #
            # schedule: 4-deep software pipeline.  Inside process(c):
            #   h1: ld(c+4); allred(c+3) [Pool, deps landed a cycle ago];
            #       red(c+1) [tiny DVE] + combine
            #   h2: thr_act(c+1); diag(c+2); neg(c+4)
            #   h3: affine(c+1)  [after the last select so the DVE never
            #       head-of-line blocks on the threshold chain]
            # Channels 0..N_OLD-1 use the DVE minmax scan instead (the Pool
            # route's latency cannot be hidden during the ramp).
            # ------------------------------------------------------------- #
            ld_scan0()
            emit_rp()
            ld(1)
            thr_act(0)
            affine(0)
            ld(2)
            ld(3)
            neg(3)

            def in_new(c):
                return N_OLD <= c < C_PER

            def mk_hooks(c):
                def h1():
                    if c + 4 < C_PER:
                        ld(c + 4)
                    if in_new(c + 3):
                        allred(c + 3)
                    if c + 1 < C_PER:
                        if c + 1 < N_OLD:
                            old_scan(c + 1)
                        else:
                            red(c + 1)

                def h2():
                    if c + 1 < C_PER:
                        thr_act(c + 1)
                    if in_new(c + 2):
                        diag(c + 2)
                    if in_new(c + 4):
                        neg(c + 4)

                def h3():
                    if c + 1 < C_PER:
                        affine(c + 1)

                return h1, h2, h3

            for c in range(C_PER):
                h1, h2, h3 = mk_hooks(c)
                process(c, hook1=h1, hook2=h2, hook3=h3)

    nc.compile()
    _CACHE["nc"] = nc
    return nc


# --------------------------------------------------------------------------- #
# Host entry point
# --------------------------------------------------------------------------- #
def kernel(x, region_percentiles, _trace=False):
    x = np.asarray(x)
    in_dtype = x.dtype
    xs = np.ascontiguousarray(x, dtype=np.float32).reshape(
        N_CORES, C_PER, P, FD
    )
    rp = np.sort(
        np.ascontiguousarray(region_percentiles, dtype=np.float32), axis=1
    ).reshape(N_CORES, C_PER, R - 1)

    nc = _build_module()
    from concourse.bass_utils import run_bass_kernel_spmd

    in_maps = [{"x": xs[i], "rp": np.ascontiguousarray(rp[i])} for i in range(N_CORES)]
    res = run_bass_kernel_spmd(
        nc, in_maps, core_ids=list(range(N_CORES)), trace=_trace
    )
    _CACHE["last_result"] = res
    y = np.stack([np.asarray(res.results[i]["y"]) for i in range(N_CORES)])
    return y.reshape(B, CC, H, W).astype(in_dtype)
